# revision 6
# baseline (speedup 1.0000x reference)
"""GATv2 3-layer GNN forward on 8 Trainium2 NeuronCores (Bass/Tile).

Strategy (edge parallelism by dst-range):
  - Edges (incl. self-loops) sorted by dst; core c owns dst nodes
    [5000c, 5000(c+1)). All segment reductions are core-local.
  - Per 127-node block, edges are processed in 128-edge tiles:
      psum_t = S_T^T @ xr_block + I @ gather(xl_table, src)   (TensorE)
      (S_T one-hot has an extra ea-value row -> folds in ea*we rank-1)
      logits = lin + sum_c 0.4*att*|t|   (lrelu = 0.6 t + 0.4 |t| split;
      the 0.6*att.t linear part comes from augmented table/xr columns)
      ex = exp(logits)  (softmax max-subtraction skipped; logits are small)
      scatter: psum_blk += S^T @ [ex*xl | ex]   (TensorE one-hot)
  - Block epilogue: h = tanh(psum_y / den + bias); next layer xl/xr
    computed by transpose+matmul; xl table AllGathered across cores.
  - Pooling: one-hot matmul into per-core partials on local graph ids,
    indirect-DMA scatter to global [512,8], AllReduce, tiny head.

Host does index/layout preprocessing only (sort, blocking, one-hot S_T
tiles, fused weight matrices); all activation compute runs on device.
"""
import sys

for _p in ("/opt/trn_rl_repo",):
    if _p not in sys.path:
        sys.path.insert(0, _p)

import numpy as np

N = 40000
E = 500000
B = 512
NC = 8
NPC = N // NC            # nodes per core
BLK = 127                # real nodes per 128-row block (row 127 = ea/we slot)
NBLK = -(-NPC // BLK)    # blocks per core (40)
PADN = NBLK * 128        # padded node rows per core (5120)
HEADS = [(8, 32), (8, 16), (1, 8)]   # (H, C) per layer
DIMS = [h * c for h, c in HEADS]     # 256, 128, 8
WIDTHS = [d + h for d, (h, c) in zip(DIMS, HEADS)]  # 264, 136, 9
F32 = "float32"

_CACHE = {}


def _padrow(n):
    """global node id (array ok) -> padded table row"""
    c, nl = np.divmod(n, NPC)
    b, r = np.divmod(nl, BLK)
    return PADN * c + 128 * b + r


def _host_preprocess(x, edge_index, edge_attr, batch):
    src = np.asarray(edge_index[0], np.int64)
    dst = np.asarray(edge_index[1], np.int64)
    ea = np.asarray(edge_attr, np.float32).reshape(-1)

    # self loops, fill_value='mean' of incoming edge_attr
    deg = np.zeros(N, np.float32)
    np.add.at(deg, dst, np.float32(1.0))
    esum = np.zeros(N, np.float32)
    np.add.at(esum, dst, ea)
    loop_attr = np.where(deg > 0, esum / np.maximum(deg, 1.0), 0.0).astype(np.float32)
    src_f = np.concatenate([src, np.arange(N, dtype=np.int64)])
    dst_f = np.concatenate([dst, np.arange(N, dtype=np.int64)])
    ea_f = np.concatenate([ea, loop_attr]).astype(np.float32)

    order = np.argsort(dst_f, kind="stable")
    src_s, dst_s, ea_s = src_f[order], dst_f[order], ea_f[order]
    src_pad = _padrow(src_s).astype(np.int32)

    # per (core, block) edge slices; dst_s sorted so use searchsorted
    bounds = np.searchsorted(dst_s, np.arange(0, N + 1, 1))
    tiles_pb = []
    for b in range(NBLK):
        mx = 0
        for c in range(NC):
            lo = bounds[min(c * NPC + b * BLK, N)]
            hi = bounds[min(c * NPC + min((b + 1) * BLK, NPC), N)]
            mx = max(mx, hi - lo)
        tiles_pb.append(-(-mx // 128))
    T = sum(tiles_pb)

    # per-core padded tile arrays
    st_all = np.zeros((NC, T, 128, 128), np.float32)   # S_T with ea row
    src_all = np.zeros((NC, T, 128), np.int32)
    dst_all = np.full((NC, T, 128), 200.0, np.float32)  # pad sentinel
    t0 = 0
    for b in range(NBLK):
        nt = tiles_pb[b]
        for c in range(NC):
            lo = bounds[min(c * NPC + b * BLK, N)]
            hi = bounds[min(c * NPC + min((b + 1) * BLK, NPC), N)]
            ne = hi - lo
            dl = (dst_s[lo:hi] - c * NPC - b * BLK).astype(np.int64)
            sl = src_pad[lo:hi]
            el = ea_s[lo:hi]
            ti = t0 + np.arange(ne) // 128
            pi = np.arange(ne) % 128
            st_all[c, ti, dl, pi] = 1.0
            st_all[c, ti, 127, pi] = el
            src_all[c, ti, pi] = sl
            dst_all[c, ti, pi] = dl.astype(np.float32)
        t0 += nt
    # SBUF layouts: [128, T]
    src_sb = np.ascontiguousarray(src_all.transpose(0, 2, 1))
    dst_sb = np.ascontiguousarray(dst_all.transpose(0, 2, 1))

    # pooling metadata
    batch = np.asarray(batch, np.int64)
    gbase = np.array([batch[c * NPC] for c in range(NC)], np.int64)
    batchloc = np.full((NC, 128, NBLK), 200.0, np.float32)
    for c in range(NC):
        bl = batch[c * NPC:(c + 1) * NPC] - gbase[c]
        assert bl.max() < 127, "graph span exceeds 127 per core"
        for b in range(NBLK):
            nn = min((b + 1) * BLK, NPC) - b * BLK
            batchloc[c, :nn, b] = bl[b * BLK: b * BLK + nn]
    POOLPAD = 768
    g_rows = np.zeros((NC, 128, 1), np.int32)
    for c in range(NC):
        g_rows[c, :, 0] = np.minimum(gbase[c] + np.arange(128), POOLPAD - 128) \
            if gbase[c] + 127 < POOLPAD else 0
        # rows >= 512 only receive pad-partition junk; ensure uniqueness
        over = gbase[c] + np.arange(128) >= B
        g_rows[c, over, 0] = B + 64 + np.arange(128)[over]
    cnt = np.bincount(batch, minlength=B).astype(np.float32)
    rcnt = (1.0 / np.maximum(cnt, 1.0)).astype(np.float32)

    return dict(tiles_pb=tiles_pb, T=T, st_all=st_all, src_sb=src_sb, dst_sb=dst_sb,
                batchloc=batchloc, g_rows=g_rows, rcnt=rcnt)


def _fuse_weights(wl, wr, we, att, H, C):
    D = H * C
    F = np.zeros((D, H), np.float32)
    for h in range(H):
        F[h * C:(h + 1) * C, h] = att[h]
    WL = np.concatenate([wl, 0.6 * (wl @ F)], axis=1).astype(np.float32)
    WR = np.concatenate([wr, 0.6 * (wr @ F)], axis=1).astype(np.float32)
    WE = np.concatenate([we, 0.6 * (we @ F)], axis=1).astype(np.float32)
    attB4 = np.zeros(D, np.float32)
    for h in range(H):
        attB4[h * C:(h + 1) * C] = 0.4 * att[h]
    return WL, WR, WE, attB4


def _host_weights(inp):
    out = {}
    W = []
    for i, (H, C) in enumerate(HEADS, start=1):
        W.append(_fuse_weights(np.asarray(inp[f"wl{i}"], np.float32),
                               np.asarray(inp[f"wr{i}"], np.float32),
                               np.asarray(inp[f"we{i}"], np.float32),
                               np.asarray(inp[f"att{i}"], np.float32), H, C))
    # layer-1 input fusion: ext = [x0, xyz, 1]; h0 = ext @ M
    M = np.zeros((5, 7), np.float32)
    M[0, :4] = np.asarray(inp["w0"], np.float32)[0]
    M[1, 4] = M[2, 5] = M[3, 6] = 1.0
    M[4, :4] = np.asarray(inp["b0"], np.float32)
    out["WL1f"] = (M @ W[0][0]).astype(np.float32)      # [5, 264]
    out["WR1f"] = (M @ W[0][1]).astype(np.float32)
    for i in (2, 3):
        out[f"WL{i}"] = W[i - 1][0]
        out[f"WR{i}"] = W[i - 1][1]
    for i in (1, 2, 3):
        H, C = HEADS[i - 1]
        D = DIMS[i - 1]
        Wd = WIDTHS[i - 1]
        out[f"weaug{i}"] = np.tile(W[i - 1][2], (1, NBLK)).astype(np.float32)  # [1, NBLK*W]
        out[f"attB4_{i}"] = np.tile(W[i - 1][3][None, :], (128, 1))            # [128, D]
        out[f"biasRep{i}"] = np.tile(np.asarray(inp[f"b{i}"], np.float32)[None, :], (128, 1))
    out["w4rep"] = np.tile(np.asarray(inp["w4"], np.float32)[:, 0][None, :], (128, 1))  # [128, 8]
    out["b4"] = float(np.asarray(inp["b4"], np.float32)[0])
    return out


def _build_x_inputs(x):
    x = np.asarray(x, np.float32)
    ext = np.concatenate([x[:, :1], x[:, 1:], np.ones((N, 1), np.float32)], 1)  # [N,5]
    extp = np.zeros((NC * PADN, 5), np.float32)
    extp[_padrow(np.arange(N))] = ext
    xt6_full = np.ascontiguousarray(extp.T)                      # [5, NC*PADN]
    xt6_own = np.ascontiguousarray(
        extp.reshape(NC, PADN, 5).transpose(0, 2, 1))            # [NC, 5, PADN]
    return xt6_full, xt6_own


def _build_program(tiles_pb, T):
    import concourse.bass as bass
    import concourse.bacc as bacc
    import concourse.mybir as mybir
    import concourse.tile as tile

    dt = mybir.dt
    f32 = dt.float32
    i32 = dt.int32
    Alu = mybir.AluOpType
    Act = mybir.ActivationFunctionType
    IOA = bass.IndirectOffsetOnAxis
    POOLPAD = 768

    nc = bacc.Bacc("TRN2", target_bir_lowering=False, debug=False, num_devices=NC)

    # ---------------- I/O ----------------
    ein = {}
    def EIN(name, shape, d=f32):
        ein[name] = nc.dram_tensor(name, list(shape), d, kind="ExternalInput")
        return ein[name]

    st_all = EIN("st_all", [T, 128, 128])
    src_sb_d = EIN("src_sb", [128, T], i32)
    dst_sb_d = EIN("dst_sb", [128, T])
    xt6_full_d = EIN("xt6_full", [5, NC * PADN])
    xt6_own_d = EIN("xt6_own", [5, PADN])
    WL1f_d = EIN("WL1f", [5, WIDTHS[0]])
    WR1f_d = EIN("WR1f", [5, WIDTHS[0]])
    WL2_d = EIN("WL2", [DIMS[0], WIDTHS[1]])
    WR2_d = EIN("WR2", [DIMS[0], WIDTHS[1]])
    WL3_d = EIN("WL3", [DIMS[1], WIDTHS[2]])
    WR3_d = EIN("WR3", [DIMS[1], WIDTHS[2]])
    weaug_d = [EIN(f"weaug{i}", [1, NBLK * WIDTHS[i - 1]]) for i in (1, 2, 3)]
    attB4_d = [EIN(f"attB4_{i}", [128, DIMS[i - 1]]) for i in (1, 2, 3)]
    biasRep_d = [EIN(f"biasRep{i}", [128, DIMS[i - 1]]) for i in (1, 2, 3)]
    iota_d = EIN("iota_row", [128, 128])
    ident_d = EIN("ident", [128, 128])
    batchloc_d = EIN("batchloc", [128, NBLK])
    g_rows_d = EIN("g_rows", [128, 1], i32)
    rcnt_d = EIN("rcnt", [128, 4])
    w4rep_d = EIN("w4rep", [128, 8])
    b4_d = EIN("b4v", [128, 1])

    out_d = nc.dram_tensor("out", [B, 1], f32, kind="ExternalOutput")

    # internal DRAM
    tables = [nc.dram_tensor(f"table{i}", [NC * PADN, WIDTHS[i - 1]], f32)
              for i in (1, 2, 3)]
    stages = [nc.dram_tensor(f"stage{i}", [PADN, WIDTHS[i - 1]], f32)
              for i in (2, 3)]
    pool_full = nc.dram_tensor("pool_full", [POOLPAD, 8], f32)
    pool_red = nc.dram_tensor("pool_red", [B, 8], f32)

    NTILE = NC * PADN // 128  # 320

    with tile.TileContext(nc) as tc:
        import contextlib
        ctx = contextlib.ExitStack()
        with ctx:
            consts = ctx.enter_context(tc.tile_pool(name="consts", bufs=1))
            meta = ctx.enter_context(tc.tile_pool(name="meta", bufs=1))
            xrp = ctx.enter_context(tc.tile_pool(name="xrp", bufs=1))
            stp = ctx.enter_context(tc.tile_pool(name="stp", bufs=4))
            gp = ctx.enter_context(tc.tile_pool(name="gp", bufs=4))
            wp = ctx.enter_context(tc.tile_pool(name="wp", bufs=3))
            sp = ctx.enter_context(tc.tile_pool(name="sp", bufs=3))
            ep = ctx.enter_context(tc.tile_pool(name="ep", bufs=3))
            pst = ctx.enter_context(tc.tile_pool(name="psum_t", bufs=2, space="PSUM"))
            psb = ctx.enter_context(tc.tile_pool(name="psum_blk", bufs=2, space="PSUM"))
            pse = ctx.enter_context(tc.tile_pool(name="psum_epi", bufs=2, space="PSUM"))
            chp = ctx.enter_context(tc.tile_pool(name="chunk", bufs=2))

            # ---- load constants ----
            def load_const(dram, shape, d=f32):
                t = consts.tile(list(shape), d, tag=dram.name + "_c")
                nc.sync.dma_start(t[:], dram[:])
                return t
            iota_t = load_const(iota_d, [128, 128])
            ident_t = load_const(ident_d, [128, 128])
            WL1f_t = load_const(WL1f_d, [5, WIDTHS[0]])
            WR1f_t = load_const(WR1f_d, [5, WIDTHS[0]])
            WL2_t = [consts.tile([128, WIDTHS[1]], f32, tag=f"wl2_{k}", name=f"wl2_{k}") for k in range(2)]
            WR2_t = [consts.tile([128, WIDTHS[1]], f32, tag=f"wr2_{k}", name=f"wr2_{k}") for k in range(2)]
            for k in range(2):
                nc.sync.dma_start(WL2_t[k][:], WL2_d[k * 128:(k + 1) * 128, :])
                nc.sync.dma_start(WR2_t[k][:], WR2_d[k * 128:(k + 1) * 128, :])
            WL3_t = load_const(WL3_d, [128, WIDTHS[2]])
            WR3_t = load_const(WR3_d, [128, WIDTHS[2]])
            attB4_t = [load_const(attB4_d[i], [128, DIMS[i]]) for i in range(3)]
            biasRep_t = [load_const(biasRep_d[i], [128, DIMS[i]]) for i in range(3)]
            batchloc_t = load_const(batchloc_d, [128, NBLK])
            g_rows_t = load_const(g_rows_d, [128, 1], i32)
            rcnt_t = load_const(rcnt_d, [128, 4])
            w4rep_t = load_const(w4rep_d, [128, 8])
            b4_t = load_const(b4_d, [128, 1])
            src_t = meta.tile([128, T], i32)
            nc.sync.dma_start(src_t[:], src_sb_d[:])
            dst_t = meta.tile([128, T], f32)
            nc.sync.dma_start(dst_t[:], dst_sb_d[:])

            # resident xr buffers (per layer); row 127 of each block = we_aug
            xr_t = [xrp.tile([128, NBLK * WIDTHS[i]], f32, tag=f"xr{i}", name=f"xr{i}") for i in range(3)]
            for i in range(3):
                nc.sync.dma_start(xr_t[i][127:128, :], weaug_d[i][:])

            # ---- preamble: build full table1 + own xr1 ----
            CH = 16  # tiles per xt6 chunk
            for ch in range(NTILE // CH):
                xchunk = chp.tile([5, CH * 128], f32, tag="xchunk")
                nc.sync.dma_start(xchunk[:], xt6_full_d[:, ch * CH * 128:(ch + 1) * CH * 128])
                for j in range(CH):
                    pt = pse.tile([128, WIDTHS[0]], f32, tag="epi_ps")
                    nc.tensor.matmul(pt[:], lhsT=xchunk[:, j * 128:(j + 1) * 128],
                                     rhs=WL1f_t[:], start=True, stop=True)
                    st = ep.tile([128, WIDTHS[0]], f32, tag="pre_sb")
                    nc.vector.tensor_copy(st[:], pt[:])
                    i = ch * CH + j
                    nc.sync.dma_start(tables[0][i * 128:(i + 1) * 128, :], st[:])
            for ch in range(NBLK // CH + 1):
                j0, j1 = ch * CH, min((ch + 1) * CH, NBLK)
                if j0 >= j1:
                    break
                xchunk = chp.tile([5, CH * 128], f32, tag="xchunk")
                nc.sync.dma_start(xchunk[:, :(j1 - j0) * 128],
                                  xt6_own_d[:, j0 * 128:j1 * 128])
                for j in range(j1 - j0):
                    b = j0 + j
                    pt = pse.tile([128, WIDTHS[0]], f32, tag="epi_ps")
                    nc.tensor.matmul(pt[:], lhsT=xchunk[:, j * 128:(j + 1) * 128],
                                     rhs=WR1f_t[:], start=True, stop=True)
                    W0 = WIDTHS[0]
                    nc.vector.tensor_copy(xr_t[0][0:127, b * W0:(b + 1) * W0], pt[0:127, :])

            # ---- layers ----
            pool_ps = psb.tile([128, 8], f32, tag="pool_ps", space="PSUM")
            for li in range(3):
                H, C = HEADS[li]
                D = DIMS[li]
                W = WIDTHS[li]
                table = tables[li]
                attB4 = attB4_t[li]
                is_last = li == 2

                if li > 0:
                    # table_li = AllGather(stage_li)
                    nc.gpsimd.collective_compute(
                        "AllGather", Alu.bypass,
                        replica_groups=[list(range(NC))],
                        ins=[stages[li - 1].ap().opt()],
                        outs=[table.ap().opt()],
                    )

                t0 = 0
                for b in range(NBLK):
                    nt = tiles_pb[b]
                    pblk = psb.tile([128, W], f32, tag="blk_ps", space="PSUM")
                    for t in range(t0, t0 + nt):
                        stile = stp.tile([128, 128], f32, tag="st_tile")
                        nc.sync.dma_start(stile[:], st_all[t])
                        g = gp.tile([128, W], f32, tag="g_tile")
                        nc.gpsimd.indirect_dma_start(
                            out=g[:], out_offset=None, in_=table[:],
                            in_offset=IOA(ap=src_t[:, t:t + 1], axis=0),
                        )
                        ptile = pst.tile([128, W], f32, tag="t_ps", space="PSUM")
                        nc.tensor.matmul(ptile[:], lhsT=stile[:],
                                         rhs=xr_t[li][:, b * W:(b + 1) * W],
                                         start=True, stop=False)
                        nc.tensor.matmul(ptile[:], lhsT=ident_t[:], rhs=g[:],
                                         start=False, stop=True)
                        u = wp.tile([128, D], f32, tag="u_tile")
                        nc.scalar.activation(u[:], ptile[:, 0:D], Act.Abs)
                        w = wp.tile([128, D], f32, tag="w_tile")
                        nc.vector.tensor_tensor(out=w[:], in0=u[:], in1=attB4[:],
                                                op=Alu.mult)
                        lg = sp.tile([128, H], f32, tag="lg")
                        nc.vector.tensor_reduce(
                            out=lg[:], in_=w[:].rearrange("p (h c) -> p h c", h=H),
                            axis=mybir.AxisListType.X, op=Alu.add)
                        nc.vector.tensor_tensor(out=lg[:], in0=lg[:],
                                                in1=ptile[:, D:W], op=Alu.add)
                        ex = sp.tile([128, H], f32, tag="ex")
                        nc.scalar.activation(ex[:], lg[:], Act.Exp)
                        y = gp.tile([128, W], f32, tag="y_tile")
                        nc.vector.tensor_tensor(
                            out=y[:, 0:D].rearrange("p (h c) -> p h c", h=H),
                            in0=g[:, 0:D].rearrange("p (h c) -> p h c", h=H),
                            in1=ex[:].unsqueeze(2).to_broadcast([128, H, C]),
                            op=Alu.mult)
                        nc.vector.tensor_copy(y[:, D:W], ex[:])
                        Smat = stp.tile([128, 128], f32, tag="s_tile")
                        nc.vector.tensor_tensor(
                            out=Smat[:], in0=dst_t[:, t:t + 1].to_broadcast([128, 128]),
                            in1=iota_t[:], op=Alu.is_equal)
                        nc.tensor.matmul(pblk[:], lhsT=Smat[:], rhs=y[:],
                                         start=(t == t0), stop=(t == t0 + nt - 1))
                    t0 += nt

                    # ---- block epilogue ----
                    den = sp.tile([128, H], f32, tag="den")
                    nc.vector.tensor_scalar_add(den[:], pblk[:, D:W], 1e-30)
                    rden = sp.tile([128, H], f32, tag="rden")
                    nc.vector.reciprocal(rden[:], den[:])
                    hr = ep.tile([128, D], f32, tag="hr")
                    nc.vector.tensor_tensor(
                        out=hr[:].rearrange("p (h c) -> p h c", h=H),
                        in0=pblk[:, 0:D].rearrange("p (h c) -> p h c", h=H),
                        in1=rden[:].unsqueeze(2).to_broadcast([128, H, C]),
                        op=Alu.mult)
                    nc.vector.tensor_tensor(out=hr[:], in0=hr[:], in1=biasRep_t[li][:],
                                            op=Alu.add)
                    h = ep.tile([128, D], f32, tag="h_blk")
                    nc.scalar.activation(h[:], hr[:], Act.Tanh)

                    if not is_last:
                        W2 = WIDTHS[li + 1]
                        WLn = [WL2_t[0], WL2_t[1]] if li == 0 else [WL3_t]
                        WRn = [WR2_t[0], WR2_t[1]] if li == 0 else [WR3_t]
                        nk = D // 128
                        hT = []
                        for k in range(nk):
                            tp = pse.tile([128, 128], f32, tag="epi_ps", space="PSUM")
                            nc.tensor.transpose(tp[:], h[:, k * 128:(k + 1) * 128],
                                                ident_t[:])
                            hTk = ep.tile([128, 128], f32, tag=f"hT{k}")
                            nc.vector.tensor_copy(hTk[:], tp[:])
                            hT.append(hTk)
                        pxl = pse.tile([128, W2], f32, tag="epi_ps", space="PSUM")
                        for k in range(nk):
                            nc.tensor.matmul(pxl[:], lhsT=hT[k][:], rhs=WLn[k][:],
                                             start=(k == 0), stop=(k == nk - 1))
                        xlout = ep.tile([128, W2], f32, tag="xlout")
                        nc.vector.tensor_copy(xlout[:], pxl[:])
                        nc.sync.dma_start(stages[li][b * 128:(b + 1) * 128, :], xlout[:])
                        pxr = pse.tile([128, W2], f32, tag="epi_ps", space="PSUM")
                        for k in range(nk):
                            nc.tensor.matmul(pxr[:], lhsT=hT[k][:], rhs=WRn[k][:],
                                             start=(k == 0), stop=(k == nk - 1))
                        nc.vector.tensor_copy(
                            xr_t[li + 1][0:127, b * W2:(b + 1) * W2], pxr[0:127, :])
                    else:
                        # pooling scatter: Sg_T[n, g] = (batchloc[n] == g)
                        Sg = stp.tile([128, 128], f32, tag="sg_tile")
                        nc.vector.tensor_tensor(
                            out=Sg[:],
                            in0=batchloc_t[:, b:b + 1].to_broadcast([128, 128]),
                            in1=iota_t[:], op=Alu.is_equal)
                        nc.tensor.matmul(pool_ps[:], lhsT=Sg[:], rhs=h[:],
                                         start=(b == 0), stop=(b == NBLK - 1))

            # ---- pooling + head ----
            pool_sb = ep.tile([128, 8], f32, tag="pool_sb")
            nc.vector.tensor_copy(pool_sb[:], pool_ps[:])
            zero8 = consts.tile([128, 8], f32, tag="zero8")
            nc.gpsimd.memset(zero8[:], 0.0)
            for i in range(POOLPAD // 128):
                nc.sync.dma_start(pool_full[i * 128:(i + 1) * 128, :], zero8[:])
            nc.gpsimd.indirect_dma_start(
                out=pool_full[:], out_offset=IOA(ap=g_rows_t[:, :1], axis=0),
                in_=pool_sb[:], in_offset=None)
            nc.gpsimd.collective_compute(
                "AllReduce", Alu.add, replica_groups=[list(range(NC))],
                ins=[pool_full.ap()[0:B, :].opt()], outs=[pool_red.ap().opt()])
            for i in range(B // 128):
                pt = ep.tile([128, 8], f32, tag="head_in")
                nc.sync.dma_start(pt[:], pool_red[i * 128:(i + 1) * 128, :])
                pw = ep.tile([128, 8], f32, tag="head_w")
                nc.vector.tensor_tensor(out=pw[:], in0=pt[:], in1=w4rep_t[:], op=Alu.mult)
                hred = ep.tile([128, 1], f32, tag="head_red")
                nc.vector.tensor_reduce(out=hred[:], in_=pw[:],
                                        axis=mybir.AxisListType.X, op=Alu.add)
                nc.vector.tensor_tensor(out=hred[:], in0=hred[:],
                                        in1=rcnt_t[:, i:i + 1], op=Alu.mult)
                nc.vector.tensor_tensor(out=hred[:], in0=hred[:], in1=b4_t[:],
                                        op=Alu.add)
                nc.sync.dma_start(out_d[i * 128:(i + 1) * 128, :], hred[:])

    nc.compile()
    return nc


def _get_program(inputs):
    pre = _host_preprocess(inputs["x"], inputs["edge_index"], inputs["edge_attr"],
                           inputs["batch"])
    key = tuple(pre["tiles_pb"])
    if key not in _CACHE:
        _CACHE[key] = _build_program(pre["tiles_pb"], pre["T"])
    return _CACHE[key], pre


def _make_in_maps(inputs, pre):
    wts = _host_weights(inputs)
    xt6_full, xt6_own = _build_x_inputs(inputs["x"])
    iota = np.tile(np.arange(128, dtype=np.float32), (128, 1))
    ident = np.eye(128, dtype=np.float32)
    in_maps = []
    for c in range(NC):
        m = dict(
            st_all=pre["st_all"][c], src_sb=pre["src_sb"][c], dst_sb=pre["dst_sb"][c],
            xt6_full=xt6_full, xt6_own=xt6_own[c],
            WL1f=wts["WL1f"], WR1f=wts["WR1f"], WL2=wts["WL2"], WR2=wts["WR2"],
            WL3=wts["WL3"], WR3=wts["WR3"],
            iota_row=iota, ident=ident,
            batchloc=pre["batchloc"][c], g_rows=pre["g_rows"][c],
            rcnt=np.ascontiguousarray(pre["rcnt"].reshape(4, 128).T),
            w4rep=wts["w4rep"], b4v=np.full((128, 1), wts["b4"], np.float32),
        )
        for i in (1, 2, 3):
            m[f"weaug{i}"] = wts[f"weaug{i}"]
            m[f"attB4_{i}"] = wts[f"attB4_{i}"]
            m[f"biasRep{i}"] = wts[f"biasRep{i}"]
        in_maps.append(m)
    return in_maps


def kernel(**inputs):
    from concourse.bass_utils import run_bass_kernel_spmd
    nc, pre = _get_program(inputs)
    in_maps = _make_in_maps(inputs, pre)
    res = run_bass_kernel_spmd(nc, in_maps, core_ids=list(range(NC)))
    return np.asarray(res.results[0]["out"], np.float32)
